# revision 11
# baseline (speedup 1.0000x reference)
"""Causal single-head self-attention on 8 Trainium2 NeuronCores.

Problem: x:[8,2048,1024], Wq/Wk/Wv:[1024,64] ->
    out[b] = softmax(tril(x[b]Wq (x[b]Wk)^T / 64)) @ (x[b]Wv)   [8,2048,64]

Sharding: data-parallel over batch -- core b gets batch element b.
Weights replicated.

Per-core algorithm (all matmul operands bf16, fp32 PSUM):
  - host pre-swizzles x[b] into xp[128, 8, 2048] (partition, e-chunk,
    seq) bf16 so every DMA is dense; kernel output is natural
    out[128, 16, 64] fp32.
  - packed projections ([Wq|Wk] -> qT@0:64,kT@64:128 and [Wk|Wq] ->
    kT@0:64,qT@64:128) give both score-matmul operand placements so
    score matmuls row-pack two-at-a-time on PE row groups 0:64/64:128.
  - v projected per 128-chunk to natural v[s,64] bf16 + ones column.
  - per pair of k-chunks: scores^T[kc, qb] = kT_kc.T @ qT_qb (K=64,
    fp32 psum); ONE exp per pair via ACT (~1ns/column + init; the
    pre-diagonal garbage columns are never consumed); causal = skip
    above-diagonal chunks + truncate diagonal chunks + gpsimd
    affine_select; out^T psum[65, qb] += v_aug[kc].T @ expT with row
    64 accumulating softmax denominators.
  - normalize: PE-transpose [65,128] slices (psum borrowed from the
    score pool), per-partition reciprocal + multiply, batched store.

Schedule: the PE runs ~2x faster after ~3us of CONTINUOUS busy
(p-state ramp), and ACT serializes one ~1.1us exp per pair, so the 20
score pairs are spread nearly uniformly across the whole kernel
instead of bunching late (attention block qb has qb+1 pairs).  Pairs
of block qb+1 are pulled into block qb's stream (po pool bufs=2 keeps
two out^T accumulators live), v projections are deferred per-chunk to
act as late-phase PE filler, and each pair carries a hand-sized chunk
of independent PE work between its score matmuls and its exp-dependent
attn@v matmuls.  Phase pair counts become 4/5/6/5 instead of 2/4/6/8.
"""

import os
from contextlib import ExitStack

import numpy as np

import concourse.bass as bass
import concourse.mybir as mybir
import concourse.tile as tile
from concourse import bacc
from concourse.bass_utils import run_bass_kernel_spmd
from concourse.masks import make_identity

B, S, E, H = 8, 2048, 1024, 64
P = 128
QB = 512  # q-block (psum free dim)
F32 = mybir.dt.float32
BF16 = mybir.dt.bfloat16


def build_kernel_body(tc, xp_d, wqk_d, wkq_d, wv_d, out_d, s=S, e_dim=E):
    nc = tc.nc
    EC = e_dim // P  # e-chunks
    NQB = s // QB    # q-blocks
    NT = s // P      # s-tiles of 128
    KPQ = QB // P    # k-chunks per q-block (4)

    ctx = ExitStack()
    with ctx:
        const = ctx.enter_context(tc.tile_pool(name="const", bufs=1))
        big = ctx.enter_context(tc.tile_pool(name="big", bufs=1))

        # weights on the ACT HWDGE ring so the Sync ring starts on x
        wqk_sb = const.tile([P, EC, 2 * H], BF16)
        nc.scalar.dma_start(wqk_sb[:], wqk_d[:])
        wkq_sb = const.tile([P, EC, 2 * H], BF16)
        nc.scalar.dma_start(wkq_sb[:], wkq_d[:])
        wv_sb = const.tile([P, EC, H], BF16)
        nc.scalar.dma_start(wv_sb[:], wv_d[:])
        ident32 = const.tile([H + 1, H + 1], F32)
        make_identity(nc, ident32[:])
        # 64-rotated identity: swap @ x exchanges partition halves
        swap_sb = const.tile([P, P], BF16)
        nc.gpsimd.memset(swap_sb[:], 0.0)
        make_identity(nc, swap_sb[0:H, H:P], nomemset=True)
        make_identity(nc, swap_sb[H:P, 0:H], nomemset=True)

        # x blocks q-block-major; first q-block split finer so the
        # first projection matmul can start after 128 KB
        xp_sb = big.tile([P, EC, s], BF16)
        for qb in range(NQB):
            if qb == 0:
                splits = [(0, 1), (1, 2), (2, 4), (4, 6), (6, 8)]
            else:
                splits = [(0, 2), (2, 4), (4, 6), (6, 8)]
            for e0, e1 in splits:
                nc.sync.dma_start(
                    xp_sb[:, e0:e1, qb * QB:(qb + 1) * QB],
                    xp_d[:, e0:e1, qb * QB:(qb + 1) * QB])

        qkT_sb = big.tile([P, s], BF16)  # rows 0:64 qT, rows 64:128 kT
        kq2_sb = big.tile([P, s], BF16)  # rows 0:64 kT, rows 64:128 qT
        v_sb = big.tile([P, NT, H + 1], BF16)  # natural v + ones col
        out_sb = big.tile([P, NT, H], F32)

        nc.gpsimd.memset(v_sb[:, :, H:H + 1], 1.0)

        # PSUM budget (8 banks): pqk 1 + pv 1 + ps 2x2 (score pairs,
        # also borrowed for normalize transposes) + po 2 = 8
        pqk = ctx.enter_context(tc.tile_pool(name="ps_qk", bufs=1, space="PSUM"))
        pv = ctx.enter_context(tc.tile_pool(name="ps_v", bufs=1, space="PSUM"))
        ps = ctx.enter_context(tc.tile_pool(name="ps_s", bufs=2, space="PSUM"))
        po = ctx.enter_context(tc.tile_pool(name="ps_o", bufs=2, space="PSUM"))
        ep = ctx.enter_context(tc.tile_pool(name="expp", bufs=4))
        sp = ctx.enter_context(tc.tile_pool(name="smalls", bufs=4))

        psum_o_pend = [None] * NQB

        def normalize_items(qb):
            """Normalize q-block qb as interleavable emit-items:
            out^T psum[65, QB] -> copy to SBUF (frees the po buffer),
            PE-transpose each [65,128] slice to [128,65] (col 64 =
            denominators) into a borrowed score-pool psum, per-partition
            reciprocal + scale, natural-layout batched store."""
            oT = sp.tile([H + 1, QB], F32, tag="oT")
            # psum_o_pend[qb] is set when pair (qb, 0) is emitted, which
            # happens after this list is built but before the copy runs
            items = [lambda: nc.vector.tensor_copy(oT[:], psum_o_pend[qb][:])]

            def mk_j(j):
                def f():
                    pt2 = ps.tile(
                        [P, 2, QB], F32, tag="sc", name="pt2s")[:, 0, 0:H + 1]
                    nc.tensor.transpose(
                        pt2[:], oT[:, j * P:(j + 1) * P], ident32[:])
                    rec = sp.tile([P, 1], F32, tag="rec")
                    nc.vector.reciprocal(rec[:], pt2[:, H:H + 1])
                    t = qb * KPQ + j
                    nc.vector.tensor_scalar_mul(
                        out_sb[:, t, :], pt2[:, 0:H], rec[:])
                return f
            for j in range(KPQ):
                items.append(mk_j(j))
            items.append(lambda: nc.sync.dma_start(
                out_d[:, qb * KPQ:(qb + 1) * KPQ, :],
                out_sb[:, qb * KPQ:(qb + 1) * KPQ, :]))
            return items

        def proj_mm_items(qb):
            """q/k packed projection [Wq|Wk] for q-block qb: 8
            emit-items; the last one also casts psum -> qkT bf16."""
            qsl = slice(qb * QB, (qb + 1) * QB)
            psum_qk = pqk.tile([P, QB], F32, tag="qk", name=f"pqk{qb}")
            items = []

            def mk_mm(ec):
                def f():
                    nc.tensor.matmul(
                        psum_qk[:], lhsT=wqk_sb[:, ec, :],
                        rhs=xp_sb[:, ec, qsl],
                        start=(ec == 0), stop=(ec == EC - 1))
                    if ec == EC - 1:
                        nc.vector.tensor_copy(qkT_sb[:, qsl], psum_qk[:])
                return f
            for ec in range(EC):
                items.append(mk_mm(ec))
            return items

        def swap_item(qb):
            """One permutation matmul turns qkT (qT@0:64, kT@64:128)
            into the swapped placement kq2 (kT@0:64, qT@64:128).
            Depends on qkT's DVE cast, so the caller spaces it a few
            PE items after proj_mm_items(qb)'s last item."""
            qsl = slice(qb * QB, (qb + 1) * QB)
            psum_kq = pqk.tile([P, QB], F32, tag="qk", name=f"pkq{qb}")

            def f():
                nc.tensor.matmul(
                    psum_kq[:], lhsT=swap_sb[:],
                    rhs=qkT_sb[:, qsl], start=True, stop=True)
                nc.vector.tensor_copy(kq2_sb[:, qsl], psum_kq[:])
            return [f]

        def v_items(t0, t1):
            """v projection for s-tiles [t0, t1): natural layout via
            small-N matmuls, 2 emit-items per tile."""
            items = []

            def mk_v(t, e0, psum_v):
                def f():
                    for ec in range(e0, e0 + EC // 2):
                        nc.tensor.matmul(
                            psum_v[:],
                            lhsT=xp_sb[:, ec, t * P:(t + 1) * P],
                            rhs=wv_sb[:, ec, :],
                            start=(ec == 0), stop=(ec == EC - 1))
                    if e0 + EC // 2 == EC:
                        nc.vector.tensor_copy(v_sb[:, t, 0:H], psum_v[:])
                return f
            for t in range(t0, t1):
                psum_v = pv.tile([P, H], F32, tag="v", name=f"psv{t}")
                items.append(mk_v(t, 0, psum_v))
                items.append(mk_v(t, EC // 2, psum_v))
            return items

        def emit_scores(qb, pr):
            """Score pair pr of attention block qb: row-packed score
            matmuls, one exp, causal masks.  Returns the state needed
            to emit the exp-dependent attn@v matmuls later."""
            qsl = slice(qb * QB, (qb + 1) * QB)
            kc0, kc1 = 2 * pr, 2 * pr + 1
            if pr == 0:
                psum_o_pend[qb] = po.tile(
                    [H + 1, QB], F32, tag="o", name=f"po{qb}")
            psum_o = psum_o_pend[qb]
            # row-packed pair: kc0 on PE rows 0:64, kc1 on rows 64:128
            o0 = max(0, kc0 * P - qb * QB)
            o1 = max(0, kc1 * P - qb * QB)
            psum_pr = ps.tile([P, 2, QB], F32, tag="sc")
            nc.tensor.matmul(
                psum_pr[:, 0, o0:],
                lhsT=kq2_sb[0:H, kc0 * P:(kc0 + 1) * P],
                rhs=qkT_sb[0:H, qsl][:, o0:],
                start=True, stop=True)
            nc.tensor.matmul(
                psum_pr[:, 1, o1:],
                lhsT=qkT_sb[H:P, kc1 * P:(kc1 + 1) * P],
                rhs=kq2_sb[H:P, qsl][:, o1:],
                start=True, stop=True)
            # one exp per pair even when the diagonal offsets differ:
            # bank 1's [o0:o1) columns hold exp(stale psum) which
            # nothing ever reads
            et = ep.tile([P, 2, QB], BF16)
            nc.scalar.activation(
                et[:, :, o0:], psum_pr[:, :, o0:],
                mybir.ActivationFunctionType.Exp, scale=1.0 / H)
            ets = []
            for i, (kc, o) in enumerate(((kc0, o0), (kc1, o1))):
                if kc * P - qb * QB >= 0:
                    # diagonal chunk: keep where q >= k (j - p >= 0)
                    nc.gpsimd.affine_select(
                        out=et[:, i, o:], in_=et[:, i, o:],
                        compare_op=mybir.AluOpType.is_ge,
                        fill=0.0, base=0,
                        channel_multiplier=-1,
                        pattern=[[1, QB - o]])
                ets.append((kc, o, et[:, i, :]))
            return (qb, psum_o, ets)

        def emit_av(state):
            qb, psum_o, ets = state
            nkc = (qb + 1) * KPQ
            for kc, o, etv in ets:
                nc.tensor.matmul(
                    psum_o[:, o:],
                    lhsT=v_sb[:, kc, :],
                    rhs=etv[:, o:],
                    start=(kc == 0), stop=(kc == nkc - 1))

        def run_schedule(schedule):
            """schedule: list of ((qb, pr), take, fillers-source) built
            by caller.  Pairs are software-pipelined: pair i's attn@v
            is emitted after pair i+1's scores AND pair i+1's filler
            chunk, so each exp has ~1.5 pair-slots of independent PE
            work in front of its consumer."""
            pending = None
            for (qb, pr), chunk in schedule:
                cur = emit_scores(qb, pr)
                for it in chunk:
                    it()
                if pending is not None:
                    emit_av(pending)
                pending = cur
            emit_av(pending)

        # ---- the schedule ----
        # plain preamble (DMA-paced): block-0 projection; v(0,1) items
        # space the swap matmul away from the qkT cast it depends on
        for it in (proj_mm_items(0) + v_items(0, 2) + swap_item(0)
                   + v_items(2, 4)):
            it()

        def chunks(fillers, takes):
            fi = 0
            out = []
            for t in takes:
                out.append(fillers[fi:fi + t])
                fi += t
            assert fi == len(fillers), (fi, len(fillers))
            return out

        pairs = []
        # ph0: blocks 0+1 front; proj(1) spread under the first exps,
        # its swap spaced by v items
        pairs += list(zip(
            [(0, 0), (0, 1), (1, 0), (1, 1)],
            chunks(proj_mm_items(1) + v_items(4, 6) + swap_item(1)
                   + v_items(6, 8),
                   [7, 6, 2, 2])))
        # ph1: rest of block 1, front of block 2; proj(2) (incl. swap)
        # must complete before (2,0)'s scores; norm(0) spaces the swap
        pairs += list(zip(
            [(1, 2), (1, 3), (2, 0), (2, 1)],
            chunks(proj_mm_items(2) + normalize_items(0) + swap_item(2),
                   [8, 7, 0, 0])))
        # ph2: rest of block 2, front of block 3; proj(3) before
        # (3,0)'s scores, norm(1) before (3,0)'s attn@v (po reuse)
        pairs += list(zip(
            [(2, 2), (2, 3), (2, 4), (2, 5),
             (3, 0), (3, 1), (3, 2), (3, 3)],
            chunks(v_items(8, 10) + proj_mm_items(3) + v_items(10, 12)
                   + swap_item(3) + normalize_items(1),
                   [5, 4, 4, 4, 2, 2, 1, 1])))
        # ph3: rest of block 3
        pairs += list(zip(
            [(3, 4), (3, 5), (3, 6), (3, 7)],
            chunks(v_items(12, 14) + v_items(14, 16) + normalize_items(2),
                   [4, 4, 3, 3])))
        run_schedule(pairs)
        for it in normalize_items(NQB - 1):
            it()


def build_bass(s=S, e_dim=E, n_cores=B):
    nc = bacc.Bacc(
        "TRN2", target_bir_lowering=False, debug=False, num_devices=n_cores)
    EC = e_dim // P
    xp_d = nc.dram_tensor("xp", [P, EC, s], BF16, kind="ExternalInput").ap()
    wqk_d = nc.dram_tensor(
        "wqk", [P, EC, 2 * H], BF16, kind="ExternalInput").ap()
    wkq_d = nc.dram_tensor(
        "wkq", [P, EC, 2 * H], BF16, kind="ExternalInput").ap()
    wv_d = nc.dram_tensor("wv", [P, EC, H], BF16, kind="ExternalInput").ap()
    out_d = nc.dram_tensor(
        "out", [P, s // P, H], F32, kind="ExternalOutput").ap()
    with tile.TileContext(nc) as tc:
        build_kernel_body(
            tc, xp_d, wqk_d, wkq_d, wv_d, out_d, s=s, e_dim=e_dim)
    nc.compile()
    return nc


_nc_cache = None


def _ensure_ntff_hook():
    """Dev-only: provide the antenv.axon_hooks shim so trace=True can
    capture NTFF profiles through libaxon_pjrt.so in this container."""
    import sys
    import types
    import ctypes
    import contextlib

    try:
        from antenv.axon_hooks import get_axon_ntff_profile_hook  # noqa
        return
    except ImportError:
        pass
    import antenv

    mod = types.ModuleType("antenv.axon_hooks")
    _h = [None]
    mod.set_axon_ntff_profile_hook = lambda h: _h.__setitem__(0, h)
    mod.get_axon_ntff_profile_hook = lambda: _h[0]
    sys.modules["antenv.axon_hooks"] = mod
    antenv.axon_hooks = mod

    so_path = "/opt/axon/libaxon_pjrt.so"
    lib = ctypes.CDLL(so_path)
    if not hasattr(lib, "axon_start_nrt_profile"):
        return
    lib.axon_start_nrt_profile.argtypes = [
        ctypes.POINTER(ctypes.c_int64), ctypes.c_size_t]
    lib.axon_start_nrt_profile.restype = ctypes.c_int64
    lib.axon_stop_nrt_profile.argtypes = [ctypes.c_char_p]
    lib.axon_stop_nrt_profile.restype = ctypes.c_int64

    @contextlib.contextmanager
    def _hook(output_dir, device_ids):
        import jax
        jax.devices()
        if device_ids:
            ids = (ctypes.c_int64 * len(device_ids))(*device_ids)
            rc = lib.axon_start_nrt_profile(ids, len(device_ids))
        else:
            rc = lib.axon_start_nrt_profile(None, 0)
        if rc != 0:
            raise RuntimeError(f"axon_start_nrt_profile rc={rc}")
        try:
            yield
        finally:
            n = lib.axon_stop_nrt_profile(str(output_dir).encode())
            print(f"profile: {n} file(s) written to {output_dir}")

    mod.set_axon_ntff_profile_hook(_hook)

    # no bucket access in this container; keep artifacts local
    import concourse.bass_utils as bu
    bu.upload_artifacts = lambda tmpdir: tmpdir


def _swizzle(a, ec, p):
    """[E, M] -> [P, EC, M] with [pp, c, m] = a[c*p + pp, m]."""
    return np.ascontiguousarray(a.reshape(ec, p, a.shape[-1]).transpose(1, 0, 2))


def kernel(x, Wq, Wk, Wv):
    global _nc_cache
    import ml_dtypes
    bf = ml_dtypes.bfloat16

    x = np.asarray(x, dtype=np.float32)
    Wq = np.asarray(Wq, dtype=np.float32)
    Wk = np.asarray(Wk, dtype=np.float32)
    Wv = np.asarray(Wv, dtype=np.float32)

    if _nc_cache is None:
        _nc_cache = build_bass()
    nc = _nc_cache

    EC = E // P
    wqk = _swizzle(np.concatenate([Wq, Wk], axis=1).astype(bf), EC, P)
    wkq = _swizzle(np.concatenate([Wk, Wq], axis=1).astype(bf), EC, P)
    wv = _swizzle(Wv.astype(bf), EC, P)
    in_maps = []
    for b in range(B):
        in_maps.append({
            "xp": _swizzle(x[b].T.astype(bf), EC, P),
            "wqk": wqk,
            "wkq": wkq,
            "wv": wv,
        })

    trace = bool(int(os.environ.get("ATTN_TRACE", "0")))
    if trace:
        _ensure_ntff_hook()
    res = run_bass_kernel_spmd(
        nc, in_maps, core_ids=list(range(B)), trace=trace)
    if trace and res.exec_time_ns is not None:
        print(f"HW exec time: {res.exec_time_ns} ns")
        kernel.last_exec_time_ns = res.exec_time_ns
        kernel.last_results = res
    # out [128, S//128, 64] per core -> [B, S, 64]
    out = np.stack(
        [np.ascontiguousarray(
            res.results[b]["out"].transpose(1, 0, 2).reshape(S, H))
         for b in range(B)],
        axis=0)
    return out
